# revision 7
# baseline (speedup 1.0000x reference)
"""ForgetMult linear recurrence h_t = f_t*x_t + (1-f_t)*h_{t-1} on 8 trn2 cores.

Sharding: batch dim B=64 split across 8 cores (8 batches/core). Per core the
C = 8*1024 = 8192 (b,h) channels are independent scans over T=1024.

All wire traffic fp16 (host pre-transposes to channel-major [C, T]; layout/
dtype prep only, all math on device). Per channel group (128 channels):
  - DMA in  f_g, x_g [128, 1024] fp16 (256KB contiguous each, SP HWDGE ring)
  - ACT: a = 1 - f  (activation Copy, scale=-1, bias=1)
  - GpSimd: b = f*x in place into the x tile
  - DVE: tensor_tensor_scan(a, b, h0[:, g]) over the full T free dim
  - DMA out h_g [128, 1024] fp16 (ACT HWDGE ring)
"""

import numpy as np

import concourse.bacc as bacc
import concourse.bass as bass
import concourse.mybir as mybir
from concourse import bass_utils
from concourse.tile import TileContext

T = 1024
B = 64
H = 1024
NCORES = 8
BS = B // NCORES  # batches per core
C = BS * H  # channels per core (independent scans)
G = 128  # channels per group == partition dim
NG = C // G  # channel groups per core

F32 = mybir.dt.float32
F16 = mybir.dt.float16


def build_program() -> bass.Bass:
    nc = bacc.Bacc(trn_type="TRN2")
    f_d = nc.dram_tensor("f", (C, T), F16, kind="ExternalInput")
    x_d = nc.dram_tensor("x", (C, T), F16, kind="ExternalInput")
    h0_d = nc.dram_tensor("h0", (G, NG), F32, kind="ExternalInput")
    y_d = nc.dram_tensor("y", (C, T), F16, kind="ExternalOutput")

    with TileContext(nc) as tc:
        with (
            tc.tile_pool(name="consts", bufs=1) as consts,
            tc.tile_pool(name="io", bufs=8) as io,
            tc.tile_pool(name="mid", bufs=4) as mid,
            tc.tile_pool(name="hpool", bufs=4) as hpool,
        ):
            h0t = consts.tile([G, NG], F32)
            nc.sync.dma_start(out=h0t[:, :], in_=h0_d[:, :])

            # DVE owns the scans (only engine with the scan op, ~3.8us per
            # group with fp32 operands). GpSimd at ~3.8us/mult ties DVE if
            # it takes all 64 b=f*x mults, so a few move to DVE (fast TT,
            # ~1.2us) to keep GpSimd just under the scan wall.
            DVE_MULTS = 5
            for g in range(NG):
                rows = slice(g * G, (g + 1) * G)
                ft = io.tile([G, T], F16, tag="f")
                xt = io.tile([G, T], F16, tag="x")
                nc.sync.dma_start(out=ft[:, :], in_=f_d[rows, :])
                nc.sync.dma_start(out=xt[:, :], in_=x_d[rows, :])
                at = mid.tile([G, T], F32, tag="a")
                nc.scalar.activation(
                    at[:, :],
                    ft[:, :],
                    mybir.ActivationFunctionType.Copy,
                    bias=1.0,
                    scale=-1.0,
                )
                bt = mid.tile([G, T], F32, tag="b")
                eng = nc.vector if g % (NG // DVE_MULTS) == 2 else nc.gpsimd
                eng.tensor_tensor(
                    out=bt[:, :],
                    in0=ft[:, :],
                    in1=xt[:, :],
                    op=mybir.AluOpType.mult,
                )
                ht = hpool.tile([G, T], F16, tag="h")
                nc.vector.tensor_tensor_scan(
                    out=ht[:, :],
                    data0=at[:, :],
                    data1=bt[:, :],
                    initial=h0t[:, g : g + 1],
                    op0=mybir.AluOpType.mult,
                    op1=mybir.AluOpType.add,
                )
                nc.scalar.dma_start(out=y_d[rows, :], in_=ht[:, :])
    if not nc.is_finalized():
        nc.finalize()
    return nc


def run(inputs: dict, trace: bool = False, tmpdir=None) -> tuple[np.ndarray, object]:
    f = np.asarray(inputs["f"], dtype=np.float32)
    x = np.asarray(inputs["x"], dtype=np.float32)
    h0 = np.asarray(inputs["hidden_init"], dtype=np.float32)

    # Host-side prep is layout/dtype only: [T, B, H] fp32 -> per-core
    # channel-major [C, T] fp16.
    ftr = f.astype(np.float16).transpose(1, 2, 0)  # (B, H, T)
    xtr = x.astype(np.float16).transpose(1, 2, 0)

    nc = build_program()
    in_maps = []
    for m in range(NCORES):
        sl = slice(m * BS, (m + 1) * BS)
        in_maps.append(
            {
                "f": np.ascontiguousarray(ftr[sl]).reshape(C, T),
                "x": np.ascontiguousarray(xtr[sl]).reshape(C, T),
                "h0": np.ascontiguousarray(h0[sl].reshape(NG, G).T),
            }
        )
    res = bass_utils.run_bass_kernel_spmd(
        nc, in_maps, core_ids=list(range(NCORES)), trace=trace, tmpdir=tmpdir
    )
    # y arrives [C, T] fp16 per core; restore [T, BS, H] fp32
    outs = [r["y"].reshape(BS, H, T).transpose(2, 0, 1) for r in res.results]
    return np.concatenate(outs, axis=1).astype(np.float32), res


def kernel(**inputs) -> np.ndarray:
    out, _ = run(inputs, trace=False)
    return out


# revision 8
# speedup vs baseline: 1.2014x; 1.2014x over previous
"""ForgetMult linear recurrence h_t = f_t*x_t + (1-f_t)*h_{t-1} on 8 trn2 cores.

Sharding: batch dim B=64 split across 8 cores (8 batches/core). Per core the
C = 8*1024 = 8192 (b,h) channels are independent scans over T=1024.

All wire traffic fp16 (host pre-transposes to channel-major [C, T]; layout/
dtype prep only, all math on device). Per channel group (128 channels):
  - DMA in  f_g, x_g [128, 1024] fp16 (256KB contiguous each, SP HWDGE ring)
  - ACT: a = 1 - f  (activation Copy, scale=-1, bias=1)
  - GpSimd: b = f*x in place into the x tile
  - DVE: tensor_tensor_scan(a, b, h0[:, g]) over the full T free dim
  - DMA out h_g [128, 1024] fp16 (ACT HWDGE ring)
"""

import numpy as np

import concourse.bacc as bacc
import concourse.bass as bass
import concourse.mybir as mybir
from concourse import bass_utils
from concourse.tile import TileContext

T = 1024
B = 64
H = 1024
NCORES = 8
BS = B // NCORES  # batches per core
C = BS * H  # channels per core (independent scans)
G = 128  # channels per group == partition dim
NG = C // G  # channel groups per core

F32 = mybir.dt.float32
F16 = mybir.dt.float16


def build_program() -> bass.Bass:
    nc = bacc.Bacc(trn_type="TRN2")
    f_d = nc.dram_tensor("f", (C, T), F16, kind="ExternalInput")
    x_d = nc.dram_tensor("x", (C, T), F16, kind="ExternalInput")
    h0_d = nc.dram_tensor("h0", (G, NG), F32, kind="ExternalInput")
    y_d = nc.dram_tensor("y", (C, T), F16, kind="ExternalOutput")

    with TileContext(nc) as tc:
        with (
            tc.tile_pool(name="consts", bufs=1) as consts,
            tc.tile_pool(name="io", bufs=12) as io,
            tc.tile_pool(name="mid", bufs=6) as mid,
            tc.tile_pool(name="hpool", bufs=6) as hpool,
        ):
            h0t = consts.tile([G, NG], F32)
            nc.sync.dma_start(out=h0t[:, :], in_=h0_d[:, :])

            for g in range(NG):
                rows = slice(g * G, (g + 1) * G)
                ft = io.tile([G, T], F16, tag="f")
                xt = io.tile([G, T], F16, tag="x")
                nc.sync.dma_start(out=ft[:, :], in_=f_d[rows, :])
                nc.sync.dma_start(out=xt[:, :], in_=x_d[rows, :])
                at = mid.tile([G, T], F16, tag="a")
                nc.scalar.activation(
                    at[:, :],
                    ft[:, :],
                    mybir.ActivationFunctionType.Copy,
                    bias=1.0,
                    scale=-1.0,
                )
                # b = f*x computed in place into the x tile
                nc.gpsimd.tensor_tensor(
                    out=xt[:, :],
                    in0=ft[:, :],
                    in1=xt[:, :],
                    op=mybir.AluOpType.mult,
                )
                ht = hpool.tile([G, T], F16, tag="h")
                nc.vector.tensor_tensor_scan(
                    out=ht[:, :],
                    data0=at[:, :],
                    data1=xt[:, :],
                    initial=h0t[:, g : g + 1],
                    op0=mybir.AluOpType.mult,
                    op1=mybir.AluOpType.add,
                )
                nc.scalar.dma_start(out=y_d[rows, :], in_=ht[:, :])
    if not nc.is_finalized():
        nc.finalize()
    return nc


def run(inputs: dict, trace: bool = False, tmpdir=None) -> tuple[np.ndarray, object]:
    f = np.asarray(inputs["f"], dtype=np.float32)
    x = np.asarray(inputs["x"], dtype=np.float32)
    h0 = np.asarray(inputs["hidden_init"], dtype=np.float32)

    # Host-side prep is layout/dtype only: [T, B, H] fp32 -> per-core
    # channel-major [C, T] fp16.
    ftr = f.astype(np.float16).transpose(1, 2, 0)  # (B, H, T)
    xtr = x.astype(np.float16).transpose(1, 2, 0)

    nc = build_program()
    in_maps = []
    for m in range(NCORES):
        sl = slice(m * BS, (m + 1) * BS)
        in_maps.append(
            {
                "f": np.ascontiguousarray(ftr[sl]).reshape(C, T),
                "x": np.ascontiguousarray(xtr[sl]).reshape(C, T),
                "h0": np.ascontiguousarray(h0[sl].reshape(NG, G).T),
            }
        )
    res = bass_utils.run_bass_kernel_spmd(
        nc, in_maps, core_ids=list(range(NCORES)), trace=trace, tmpdir=tmpdir
    )
    # y arrives [C, T] fp16 per core; restore [T, BS, H] fp32
    outs = [r["y"].reshape(BS, H, T).transpose(2, 0, 1) for r in res.results]
    return np.concatenate(outs, axis=1).astype(np.float32), res


def kernel(**inputs) -> np.ndarray:
    out, _ = run(inputs, trace=False)
    return out


# revision 9
# speedup vs baseline: 1.2050x; 1.0030x over previous
"""ForgetMult linear recurrence h_t = f_t*x_t + (1-f_t)*h_{t-1} on 8 trn2 cores.

Sharding: batch dim B=64 split across 8 cores (8 batches/core). Per core the
C = 8*1024 = 8192 (b,h) channels are independent scans over T=1024.

All wire traffic fp16 (host pre-transposes to channel-major [C, T]; layout/
dtype prep only, all math on device). Per channel group (128 channels):
  - DMA in  f_g, x_g [128, 1024] fp16 (256KB contiguous each, SP HWDGE ring)
  - ACT: a = 1 - f  (activation Copy, scale=-1, bias=1)
  - GpSimd: b = f*x in place into the x tile
  - DVE: tensor_tensor_scan(a, b, h0[:, g]) over the full T free dim
  - DMA out h_g [128, 1024] fp16 (ACT HWDGE ring)
"""

import numpy as np

import concourse.bacc as bacc
import concourse.bass as bass
import concourse.mybir as mybir
from concourse import bass_utils
from concourse.tile import TileContext

T = 1024
B = 64
H = 1024
NCORES = 8
BS = B // NCORES  # batches per core
C = BS * H  # channels per core (independent scans)
G = 128  # channels per group == partition dim
NG = C // G  # channel groups per core

F32 = mybir.dt.float32
F16 = mybir.dt.float16


def build_program() -> bass.Bass:
    nc = bacc.Bacc(trn_type="TRN2")
    f_d = nc.dram_tensor("f", (C, T), F16, kind="ExternalInput")
    x_d = nc.dram_tensor("x", (C, T), F16, kind="ExternalInput")
    h0_d = nc.dram_tensor("h0", (G, NG), F32, kind="ExternalInput")
    y_d = nc.dram_tensor("y", (C, T), F16, kind="ExternalOutput")

    with TileContext(nc) as tc:
        with (
            tc.tile_pool(name="consts", bufs=1) as consts,
            tc.tile_pool(name="io", bufs=12) as io,
            tc.tile_pool(name="mid", bufs=6) as mid,
            tc.tile_pool(name="hpool", bufs=6) as hpool,
        ):
            h0t = consts.tile([G, NG], F32)
            nc.sync.dma_start(out=h0t[:, :], in_=h0_d[:, :])

            COPY = mybir.ActivationFunctionType.Copy
            MULT = mybir.AluOpType.mult
            ADD = mybir.AluOpType.add
            HALF = T // 2
            for g in range(NG):
                rows = slice(g * G, (g + 1) * G)
                ft = io.tile([G, T], F16, tag="f")
                xt = io.tile([G, T], F16, tag="x")
                at = mid.tile([G, T], F16, tag="a")
                ht = hpool.tile([G, T], F16, tag="h")
                if g == 0:
                    # Ramp: the first scan gates the whole DVE-bound kernel.
                    # Feed it in T-halves, with the b-mult on the (idle) DVE
                    # instead of waiting ~4us for GpSimd.
                    for i in range(2):
                        tl = slice(i * HALF, (i + 1) * HALF)
                        nc.sync.dma_start(out=ft[:, tl], in_=f_d[rows, tl])
                        nc.sync.dma_start(out=xt[:, tl], in_=x_d[rows, tl])
                        nc.scalar.activation(
                            at[:, tl], ft[:, tl], COPY, bias=1.0, scale=-1.0
                        )
                        nc.vector.tensor_tensor(
                            out=xt[:, tl], in0=ft[:, tl], in1=xt[:, tl], op=MULT
                        )
                        init = (
                            h0t[:, g : g + 1] if i == 0
                            else ht[:, HALF - 1 : HALF]
                        )
                        nc.vector.tensor_tensor_scan(
                            out=ht[:, tl], data0=at[:, tl], data1=xt[:, tl],
                            initial=init, op0=MULT, op1=ADD,
                        )
                    nc.scalar.dma_start(out=y_d[rows, :], in_=ht[:, :])
                    continue
                nc.sync.dma_start(out=ft[:, :], in_=f_d[rows, :])
                nc.sync.dma_start(out=xt[:, :], in_=x_d[rows, :])
                nc.scalar.activation(
                    at[:, :], ft[:, :], COPY, bias=1.0, scale=-1.0
                )
                # b = f*x computed in place into the x tile
                nc.gpsimd.tensor_tensor(
                    out=xt[:, :], in0=ft[:, :], in1=xt[:, :], op=MULT
                )
                if g == NG - 1:
                    # Tail: scan in halves so the first half's output DMA
                    # overlaps the second half's scan.
                    for i in range(2):
                        tl = slice(i * HALF, (i + 1) * HALF)
                        init = (
                            h0t[:, g : g + 1] if i == 0
                            else ht[:, HALF - 1 : HALF]
                        )
                        nc.vector.tensor_tensor_scan(
                            out=ht[:, tl], data0=at[:, tl], data1=xt[:, tl],
                            initial=init, op0=MULT, op1=ADD,
                        )
                        nc.scalar.dma_start(out=y_d[rows, tl], in_=ht[:, tl])
                    continue
                nc.vector.tensor_tensor_scan(
                    out=ht[:, :], data0=at[:, :], data1=xt[:, :],
                    initial=h0t[:, g : g + 1],
                    op0=MULT, op1=ADD,
                )
                nc.scalar.dma_start(out=y_d[rows, :], in_=ht[:, :])
    if not nc.is_finalized():
        nc.finalize()
    return nc


def run(inputs: dict, trace: bool = False, tmpdir=None) -> tuple[np.ndarray, object]:
    f = np.asarray(inputs["f"], dtype=np.float32)
    x = np.asarray(inputs["x"], dtype=np.float32)
    h0 = np.asarray(inputs["hidden_init"], dtype=np.float32)

    # Host-side prep is layout/dtype only: [T, B, H] fp32 -> per-core
    # channel-major [C, T] fp16.
    ftr = f.astype(np.float16).transpose(1, 2, 0)  # (B, H, T)
    xtr = x.astype(np.float16).transpose(1, 2, 0)

    nc = build_program()
    in_maps = []
    for m in range(NCORES):
        sl = slice(m * BS, (m + 1) * BS)
        in_maps.append(
            {
                "f": np.ascontiguousarray(ftr[sl]).reshape(C, T),
                "x": np.ascontiguousarray(xtr[sl]).reshape(C, T),
                "h0": np.ascontiguousarray(h0[sl].reshape(NG, G).T),
            }
        )
    res = bass_utils.run_bass_kernel_spmd(
        nc, in_maps, core_ids=list(range(NCORES)), trace=trace, tmpdir=tmpdir
    )
    # y arrives [C, T] fp16 per core; restore [T, BS, H] fp32
    outs = [r["y"].reshape(BS, H, T).transpose(2, 0, 1) for r in res.results]
    return np.concatenate(outs, axis=1).astype(np.float32), res


def kernel(**inputs) -> np.ndarray:
    out, _ = run(inputs, trace=False)
    return out


# revision 10
# speedup vs baseline: 1.2478x; 1.0355x over previous
"""ForgetMult linear recurrence h_t = f_t*x_t + (1-f_t)*h_{t-1} on 8 trn2 cores.

Sharding: batch dim B=64 split across 8 cores (8 batches/core). Per core the
C = 8*1024 = 8192 (b,h) channels are independent scans over T=1024.

All wire traffic fp16 (host pre-transposes to channel-major [C, T]; layout/
dtype prep only, all math on device). Per channel group (128 channels):
  - DMA in  f_g, x_g [128, 1024] fp16 (256KB contiguous each, SP HWDGE ring)
  - ACT: a = 1 - f  (activation Copy, scale=-1, bias=1)
  - GpSimd: b = f*x in place into the x tile
  - DVE: tensor_tensor_scan(a, b, h0[:, g]) over the full T free dim
  - DMA out h_g [128, 1024] fp16 (ACT HWDGE ring)
"""

import numpy as np

import concourse.bacc as bacc
import concourse.bass as bass
import concourse.mybir as mybir
from concourse import bass_utils
from concourse.tile import TileContext

T = 1024
B = 64
H = 1024
NCORES = 8
BS = B // NCORES  # batches per core
C = BS * H  # channels per core (independent scans)
G = 128  # channels per group == partition dim
NG = C // G  # channel groups per core

F32 = mybir.dt.float32
F16 = mybir.dt.float16


def build_program() -> bass.Bass:
    nc = bacc.Bacc(trn_type="TRN2")
    f_d = nc.dram_tensor("f", (C, T), F16, kind="ExternalInput")
    x_d = nc.dram_tensor("x", (C, T), F16, kind="ExternalInput")
    h0_d = nc.dram_tensor("h0", (G, NG), F32, kind="ExternalInput")
    y_d = nc.dram_tensor("y", (C, T), F16, kind="ExternalOutput")

    with TileContext(nc) as tc:
        with (
            tc.tile_pool(name="consts", bufs=1) as consts,
            tc.tile_pool(name="io", bufs=12) as io,
            tc.tile_pool(name="mid", bufs=6) as mid,
            tc.tile_pool(name="hpool", bufs=6) as hpool,
        ):
            h0t = consts.tile([G, NG], F32)
            nc.sync.dma_start(out=h0t[:, :], in_=h0_d[:, :])

            COPY = mybir.ActivationFunctionType.Copy
            MULT = mybir.AluOpType.mult
            ADD = mybir.AluOpType.add
            HALF = T // 2
            for g in range(NG):
                rows = slice(g * G, (g + 1) * G)
                ft = io.tile([G, T], F16, tag="f")
                xt = io.tile([G, T], F16, tag="x")
                at = mid.tile([G, T], F16, tag="a")
                ht = hpool.tile([G, T], F16, tag="h")
                if g == 0:
                    # Ramp: the first scan gates the whole DVE-bound kernel.
                    # Feed it in T-halves, with the b-mult on the (idle) DVE
                    # instead of waiting ~4us for GpSimd.
                    for i in range(2):
                        tl = slice(i * HALF, (i + 1) * HALF)
                        nc.sync.dma_start(out=ft[:, tl], in_=f_d[rows, tl])
                        nc.sync.dma_start(out=xt[:, tl], in_=x_d[rows, tl])
                        nc.scalar.activation(
                            at[:, tl], ft[:, tl], COPY, bias=1.0, scale=-1.0
                        )
                        nc.vector.tensor_tensor(
                            out=xt[:, tl], in0=ft[:, tl], in1=xt[:, tl], op=MULT
                        )
                        init = (
                            h0t[:, g : g + 1] if i == 0
                            else ht[:, HALF - 1 : HALF]
                        )
                        nc.vector.tensor_tensor_scan(
                            out=ht[:, tl], data0=at[:, tl], data1=xt[:, tl],
                            initial=init, op0=MULT, op1=ADD,
                        )
                    nc.scalar.dma_start(out=y_d[rows, :], in_=ht[:, :])
                    continue
                nc.sync.dma_start(out=ft[:, :], in_=f_d[rows, :])
                nc.sync.dma_start(out=xt[:, :], in_=x_d[rows, :])
                nc.scalar.activation(
                    at[:, :], ft[:, :], COPY, bias=1.0, scale=-1.0
                )
                # b = f*x in place into the x tile. The 512-chunk scans cut
                # DVE to ~163us, which makes GpSimd's 64 mults (~257us) the
                # wall — so ~19 groups' mults run on the now-free DVE.
                meng = nc.vector if g % 10 in (1, 4, 7) else nc.gpsimd
                meng.tensor_tensor(
                    out=xt[:, :], in0=ft[:, :], in1=xt[:, :], op=MULT
                )
                # Scan in chained chunks: the scan instruction runs at
                # ~2.4 cyc/elem for 512-long chunks vs ~3.9 for 1024.
                # Chaining via the last output column is exact (one fp16
                # rounding of the carry per boundary). g==32/33 probe other
                # chunk shapes for the profile.
                if g == 32:
                    bounds = [0, 256, 512, 768, T]
                elif g == 33:
                    bounds = [0, 768, T]
                else:
                    bounds = [0, HALF, T]
                for i in range(len(bounds) - 1):
                    tl = slice(bounds[i], bounds[i + 1])
                    init = (
                        h0t[:, g : g + 1] if i == 0
                        else ht[:, bounds[i] - 1 : bounds[i]]
                    )
                    nc.vector.tensor_tensor_scan(
                        out=ht[:, tl], data0=at[:, tl], data1=xt[:, tl],
                        initial=init, op0=MULT, op1=ADD,
                    )
                    if g == NG - 1:
                        # Tail: the first chunk's output DMA overlaps the
                        # second chunk's scan.
                        nc.scalar.dma_start(out=y_d[rows, tl], in_=ht[:, tl])
                if g != NG - 1:
                    nc.scalar.dma_start(out=y_d[rows, :], in_=ht[:, :])
    if not nc.is_finalized():
        nc.finalize()
    return nc


def run(inputs: dict, trace: bool = False, tmpdir=None) -> tuple[np.ndarray, object]:
    f = np.asarray(inputs["f"], dtype=np.float32)
    x = np.asarray(inputs["x"], dtype=np.float32)
    h0 = np.asarray(inputs["hidden_init"], dtype=np.float32)

    # Host-side prep is layout/dtype only: [T, B, H] fp32 -> per-core
    # channel-major [C, T] fp16.
    ftr = f.astype(np.float16).transpose(1, 2, 0)  # (B, H, T)
    xtr = x.astype(np.float16).transpose(1, 2, 0)

    nc = build_program()
    in_maps = []
    for m in range(NCORES):
        sl = slice(m * BS, (m + 1) * BS)
        in_maps.append(
            {
                "f": np.ascontiguousarray(ftr[sl]).reshape(C, T),
                "x": np.ascontiguousarray(xtr[sl]).reshape(C, T),
                "h0": np.ascontiguousarray(h0[sl].reshape(NG, G).T),
            }
        )
    res = bass_utils.run_bass_kernel_spmd(
        nc, in_maps, core_ids=list(range(NCORES)), trace=trace, tmpdir=tmpdir
    )
    # y arrives [C, T] fp16 per core; restore [T, BS, H] fp32
    outs = [r["y"].reshape(BS, H, T).transpose(2, 0, 1) for r in res.results]
    return np.concatenate(outs, axis=1).astype(np.float32), res


def kernel(**inputs) -> np.ndarray:
    out, _ = run(inputs, trace=False)
    return out


# revision 14
# speedup vs baseline: 1.6250x; 1.3023x over previous
"""ForgetMult linear recurrence h_t = f_t*x_t + (1-f_t)*h_{t-1} on 8 trn2 cores.

Sharding: batch dim B=64 split across 8 cores (8 batches/core). Per core the
C = 8*1024 = 8192 (b,h) channels are independent scans over T=1024.

All wire traffic fp16 (host pre-transposes to channel-major [C, T]; layout/
dtype prep only, all math on device). Per channel group (128 channels):
  - DMA in  f_g, x_g [128, 1024] fp16 (256KB contiguous each, SP HWDGE ring)
  - ACT: a = 1 - f  (activation Copy, scale=-1, bias=1)
  - GpSimd: b = f*x in place into the x tile
  - DVE: tensor_tensor_scan(a, b, h0[:, g]) over the full T free dim
  - DMA out h_g [128, 1024] fp16 (ACT HWDGE ring)
"""

import numpy as np

import concourse.bacc as bacc
import concourse.bass as bass
import concourse.mybir as mybir
from concourse import bass_utils
from concourse.tile import TileContext

T = 1024
B = 64
H = 1024
NCORES = 8
BS = B // NCORES  # batches per core
C = BS * H  # channels per core (independent scans)
G = 128  # channels per group == partition dim
NG = C // G  # channel groups per core

F32 = mybir.dt.float32
F16 = mybir.dt.float16


def build_program() -> bass.Bass:
    nc = bacc.Bacc(trn_type="TRN2")
    f_d = nc.dram_tensor("f", (C, T), F16, kind="ExternalInput")
    x_d = nc.dram_tensor("x", (C, T), F16, kind="ExternalInput")
    h0_d = nc.dram_tensor("h0", (G, NG), F32, kind="ExternalInput")
    y_d = nc.dram_tensor("y", (C, T), F16, kind="ExternalOutput")

    with TileContext(nc) as tc:
        with (
            tc.tile_pool(name="consts", bufs=1) as consts,
            tc.tile_pool(name="io", bufs=12) as io,
            tc.tile_pool(name="mid", bufs=6) as mid,
            tc.tile_pool(name="hpool", bufs=6) as hpool,
        ):
            h0t = consts.tile([G, NG], F32)
            nc.sync.dma_start(out=h0t[:, :], in_=h0_d[:, :])

            COPY = mybir.ActivationFunctionType.Copy
            MULT = mybir.AluOpType.mult
            ADD = mybir.AluOpType.add
            HALF = T // 2
            for g in range(NG):
                rows = slice(g * G, (g + 1) * G)
                ft = io.tile([G, T], F16, tag="f")
                xt = io.tile([G, T], F16, tag="x")
                at = mid.tile([G, T], F16, tag="a")
                ht = hpool.tile([G, T], F16, tag="h")
                if g == 0:
                    # Ramp: the first scan gates the whole DVE-bound kernel.
                    # Feed it in T-halves, with the b-mult on the (idle) DVE
                    # instead of waiting ~4us for GpSimd.
                    for i in range(2):
                        tl = slice(i * HALF, (i + 1) * HALF)
                        nc.sync.dma_start(out=ft[:, tl], in_=f_d[rows, tl])
                        nc.sync.dma_start(out=xt[:, tl], in_=x_d[rows, tl])
                        nc.scalar.activation(
                            at[:, tl], ft[:, tl], COPY, bias=1.0, scale=-1.0
                        )
                        bt0 = mid.tile([G, T], F16, tag="b", name=f"b0_{i}")
                        nc.vector.tensor_tensor(
                            out=bt0[:, tl], in0=ft[:, tl], in1=xt[:, tl], op=MULT
                        )
                        init = (
                            h0t[:, g : g + 1] if i == 0
                            else ht[:, HALF - 1 : HALF]
                        )
                        nc.vector.tensor_tensor_scan(
                            out=ht[:, tl], data0=at[:, tl], data1=bt0[:, tl],
                            initial=init, op0=MULT, op1=ADD,
                        )
                    nc.scalar.dma_start(out=y_d[rows, :], in_=ht[:, :])
                    continue
                nc.sync.dma_start(out=ft[:, :], in_=f_d[rows, :])
                nc.sync.dma_start(out=xt[:, :], in_=x_d[rows, :])
                nc.scalar.activation(
                    at[:, :], ft[:, :], COPY, bias=1.0, scale=-1.0
                )
                # Concurrent GpSimd SBUF traffic slows DVE scans ~2.3x (SBUF
                # port contention), so DVE does everything itself: b = f*x
                # into a separate tile (2x-mode needs non-in-place), then the
                # scan at the quiet rate (~230cyc + 2.1 cyc/elem).
                bt = mid.tile([G, T], F16, tag="b")
                nc.vector.tensor_tensor(
                    out=bt[:, :], in0=ft[:, :], in1=xt[:, :], op=MULT
                )
                xt = bt
                # g==32/33 probe chunked-scan shapes at the quiet rate.
                if g in (32, NG - 1):
                    bounds = [0, HALF, T]
                elif g == 33:
                    bounds = [0, 256, T]
                else:
                    bounds = [0, T]
                for i in range(len(bounds) - 1):
                    tl = slice(bounds[i], bounds[i + 1])
                    init = (
                        h0t[:, g : g + 1] if i == 0
                        else ht[:, bounds[i] - 1 : bounds[i]]
                    )
                    nc.vector.tensor_tensor_scan(
                        out=ht[:, tl], data0=at[:, tl], data1=xt[:, tl],
                        initial=init, op0=MULT, op1=ADD,
                    )
                    if g == NG - 1:
                        # Tail: the first chunk's output DMA overlaps the
                        # second chunk's scan.
                        nc.scalar.dma_start(out=y_d[rows, tl], in_=ht[:, tl])
                if g != NG - 1:
                    nc.scalar.dma_start(out=y_d[rows, :], in_=ht[:, :])
    if not nc.is_finalized():
        nc.finalize()
    return nc


def run(inputs: dict, trace: bool = False, tmpdir=None) -> tuple[np.ndarray, object]:
    f = np.asarray(inputs["f"], dtype=np.float32)
    x = np.asarray(inputs["x"], dtype=np.float32)
    h0 = np.asarray(inputs["hidden_init"], dtype=np.float32)

    # Host-side prep is layout/dtype only: [T, B, H] fp32 -> per-core
    # channel-major [C, T] fp16.
    ftr = f.astype(np.float16).transpose(1, 2, 0)  # (B, H, T)
    xtr = x.astype(np.float16).transpose(1, 2, 0)

    nc = build_program()
    in_maps = []
    for m in range(NCORES):
        sl = slice(m * BS, (m + 1) * BS)
        in_maps.append(
            {
                "f": np.ascontiguousarray(ftr[sl]).reshape(C, T),
                "x": np.ascontiguousarray(xtr[sl]).reshape(C, T),
                "h0": np.ascontiguousarray(h0[sl].reshape(NG, G).T),
            }
        )
    res = bass_utils.run_bass_kernel_spmd(
        nc, in_maps, core_ids=list(range(NCORES)), trace=trace, tmpdir=tmpdir
    )
    # y arrives [C, T] fp16 per core; restore [T, BS, H] fp32
    outs = [r["y"].reshape(BS, H, T).transpose(2, 0, 1) for r in res.results]
    return np.concatenate(outs, axis=1).astype(np.float32), res


def kernel(**inputs) -> np.ndarray:
    out, _ = run(inputs, trace=False)
    return out
